# revision 1
# baseline (speedup 1.0000x reference)
"""Trainium2 Bass kernel for LlamaRALAAttention (B=2, S=4096, HID=2048, NH=16, NKV=4, HD=128).

Sharding: 8 cores = DP(batch=2) x TP(kv-head groups=4). Core c handles batch c//4,
kv group c%4 (4 q heads + 1 kv head). Softmax/mean over S stay core-local.
o_proj partials are summed on host (the only cross-core reduction).

Pipeline (per core, "everything transposed" layout):
  xT [HID,S] host-pretransposed, bf16. Projections stream xT chunks as moving operand.
  q path in [d,s] layout: q^T = Wq_h^T @ xT, RoPE via R-matmul + cos/sin mults,
    kappa=exp(min(x,0))+max(x,0) -> QkT (bf16, resident).
  k/v path in [s,d] layout: lhsT=xT tile (stationary), rhs=[Wk|Wv]; RoPE on free dim;
    kappa -> Kk_sd, v_sd (bf16, resident). KkT via PE transpose.
  Qg = mean_s Qk (DVE free-dim reduce); logits via per-s-tile matvecs (lhsT=KkT tile);
  softmax with exact global max (PE transpose + ones-matmul broadcasts, all on-chip);
  outer = (alpha*Kk)^T @ v (PE accumulate); result^T = outer^T.T... lhsT=outer, rhs=QkT;
  ctx^T = phiT * result^T; o_proj: lhsT=ctx^T tiles, rhs=Wo rows -> partial out [S, 2048].
"""

import sys

sys.path.insert(0, "/opt/trn_rl_repo")

import numpy as np
import ml_dtypes

import concourse.bass as bass
import concourse.mybir as mybir
import concourse.tile as tile
from concourse import bacc
from concourse.bass_utils import run_bass_kernel_spmd
from concourse.masks import make_identity

P = 128
S = 4096
HID = 2048
HD = 128
NHL = 4            # q heads per core
KO = HID // P      # 16 contraction subtiles
CS = 512           # token chunk size
NCH = S // CS      # 8 chunks
NST = S // P       # 32 s-tiles
ROPE_THETA = 10000.0

F32 = mybir.dt.float32
BF16 = mybir.dt.bfloat16
BF = ml_dtypes.bfloat16

_CACHE = {}


def _build():
    nc = bacc.Bacc("TRN2", target_bir_lowering=False, debug=False, num_devices=8)

    xT = nc.dram_tensor("xT", [HID, S], BF16, kind="ExternalInput").ap()
    cosT = nc.dram_tensor("cosT", [P, S], F32, kind="ExternalInput").ap()
    sinT = nc.dram_tensor("sinT", [P, S], F32, kind="ExternalInput").ap()
    cos_sd = nc.dram_tensor("cos_sd", [S, HD], F32, kind="ExternalInput").ap()
    sin_sd = nc.dram_tensor("sin_sd", [S, HD], F32, kind="ExternalInput").ap()
    Wq = nc.dram_tensor("Wq", [HID, NHL * HD], BF16, kind="ExternalInput").ap()
    Wkv = nc.dram_tensor("Wkv", [HID, 2 * HD], BF16, kind="ExternalInput").ap()
    Wphi = nc.dram_tensor("Wphi", [HID, NHL * HD], BF16, kind="ExternalInput").ap()
    Wo = nc.dram_tensor("Wo", [NHL * HD, HID], BF16, kind="ExternalInput").ap()
    bphi = nc.dram_tensor("bphi", [NHL * HD], F32, kind="ExternalInput").ap()
    RT = nc.dram_tensor("RT", [P, P], BF16, kind="ExternalInput").ap()
    out = nc.dram_tensor("out", [S, HID], F32, kind="ExternalOutput").ap()

    xT_r = xT.rearrange("(ko p) s -> p ko s", p=P)
    Wq_r = Wq.rearrange("(ko p) m -> p ko m", p=P)
    Wkv_r = Wkv.rearrange("(ko p) m -> p ko m", p=P)
    Wphi_r = Wphi.rearrange("(ko p) m -> p ko m", p=P)
    Wo_r = Wo.rearrange("(h p) n -> p h n", p=P)
    cos_sd_r = cos_sd.rearrange("(t p) d -> p t d", p=P)
    sin_sd_r = sin_sd.rearrange("(t p) d -> p t d", p=P)
    bphi_r = bphi.rearrange("(h p) -> p h", p=P)
    out_r = out.rearrange("(t p) n -> p t n", p=P)

    from contextlib import ExitStack
    with tile.TileContext(nc) as tc, ExitStack() as es:
        # ---- pools ----
        res = es.enter_context(tc.tile_pool(name="res", bufs=1))        # residents
        wts = es.enter_context(tc.tile_pool(name="wts", bufs=2))        # big weights, shared slots
        xp = es.enter_context(tc.tile_pool(name="xp", bufs=3))          # xT chunks
        stream = es.enter_context(tc.tile_pool(name="stream", bufs=2))  # big per-chunk tiles
        stream3 = es.enter_context(tc.tile_pool(name="stream3", bufs=3))  # small per-chunk tiles
        small = es.enter_context(tc.tile_pool(name="small", bufs=4))    # tiny tiles
        pq = es.enter_context(tc.tile_pool(name="pq", bufs=3, space="PSUM"))    # [128,512] proj
        pr = es.enter_context(tc.tile_pool(name="pr", bufs=1, space="PSUM"))    # [128,512] rot/result
        po = es.enter_context(tc.tile_pool(name="po", bufs=2, space="PSUM"))    # [128,512] out
        pmix = es.enter_context(tc.tile_pool(name="pmix", bufs=2, space="PSUM"))  # shared small

        # ---- residents / weights ----
        Wkv_sb = res.tile([P, KO, 2 * HD], BF16)
        nc.sync.dma_start(Wkv_sb[:], Wkv_r)
        Wq_sb = wts.tile([P, KO, NHL * HD], BF16, tag="big")
        RT_sb = res.tile([P, P], BF16)
        nc.sync.dma_start(RT_sb[:], RT)
        bphi_sb = res.tile([P, NHL], F32)
        nc.sync.dma_start(bphi_sb[:], bphi_r)

        ident_bf = res.tile([P, P], BF16)
        make_identity(nc, ident_bf[:])
        ident_f32 = res.tile([P, P], F32)
        make_identity(nc, ident_f32[:])
        ones_f32 = res.tile([P, 1], F32)
        nc.vector.memset(ones_f32[:], 1.0)
        onesr_f32 = res.tile([1, P], F32)
        nc.vector.memset(onesr_f32[:], 1.0)
        negr_f32 = res.tile([1, P], F32)
        nc.vector.memset(negr_f32[:], -1.0)

        QkT = res.tile([P, NHL, S], BF16)       # 32KB/part
        KkT = res.tile([P, S], BF16)            # 8KB/part
        Kk_sd = res.tile([P, NST, HD], BF16)    # 8KB/part
        v_sd = res.tile([P, NST, HD], BF16)     # 8KB/part
        qg_parts = res.tile([P, NHL, NCH], F32)
        outer_bf = res.tile([P, NHL, HD], BF16)
        alpha_sd = res.tile([P, NHL, NST], F32)
        logits_sd = res.tile([P, NHL, NST], F32)

        # ================= phase A: q/k/v projections + rope + kappa =================
        for c in range(NCH):
            xt = xp.tile([P, KO, CS], BF16, tag="xt")
            nc.sync.dma_start(xt[:], xT_r[:, :, c * CS:(c + 1) * CS])
            cs_t = stream.tile([P, CS], F32, tag="cosT")
            nc.sync.dma_start(cs_t[:], cosT[:, c * CS:(c + 1) * CS])
            sn_t = stream.tile([P, CS], F32, tag="sinT")
            nc.sync.dma_start(sn_t[:], sinT[:, c * CS:(c + 1) * CS])
            csd = stream.tile([P, 4, HD], F32, tag="cossd")
            nc.sync.dma_start(csd[:], cos_sd_r[:, c * 4:(c + 1) * 4, :])
            ssd = stream.tile([P, 4, HD], F32, tag="sinsd")
            nc.sync.dma_start(ssd[:], sin_sd_r[:, c * 4:(c + 1) * 4, :])

            # ---- k + v for the 4 s-tiles of this chunk ----
            for st in range(4):
                stg = c * 4 + st
                pskv = pmix.tile([P, 2 * HD], F32, tag="mix")
                for ko in range(KO):
                    nc.tensor.matmul(
                        pskv[:], xt[:, ko, st * P:(st + 1) * P], Wkv_sb[:, ko, :],
                        start=(ko == 0), stop=(ko == KO - 1))
                k_ps = pskv[:, :HD]
                nc.vector.tensor_copy(v_sd[:, stg, :], pskv[:, HD:])
                # rope-k in [s,d]: rot on free halves
                kr = stream3.tile([P, HD], F32, tag="kr")
                nc.vector.tensor_mul(kr[:], k_ps, csd[:, st, :])
                ta = stream3.tile([P, 64], F32, tag="ta")
                nc.vector.tensor_mul(ta[:], k_ps[:, 64:], ssd[:, st, :64])
                nc.vector.tensor_sub(kr[:, :64], kr[:, :64], ta[:])
                tb = stream3.tile([P, 64], F32, tag="tb")
                nc.vector.tensor_mul(tb[:], k_ps[:, :64], ssd[:, st, 64:])
                nc.vector.tensor_add(kr[:, 64:], kr[:, 64:], tb[:])
                # kappa
                mk = stream3.tile([P, HD], F32, tag="mk")
                nc.gpsimd.tensor_scalar_min(mk[:], kr[:], 0.0)
                ek = stream3.tile([P, HD], F32, tag="ek")
                nc.scalar.activation(ek[:], mk[:], mybir.ActivationFunctionType.Exp)
                nc.vector.scalar_tensor_tensor(
                    Kk_sd[:, stg, :], kr[:], 0.0, ek[:],
                    mybir.AluOpType.max, mybir.AluOpType.add)
                # KkT via PE transpose (bf16 in -> fp32 psum -> bf16 sbuf)
                pst = pmix.tile([P, P], BF16, tag="mix")
                nc.tensor.transpose(pst[:], Kk_sd[:, stg, :], ident_bf[:])
                nc.vector.tensor_copy(KkT[:, stg * P:(stg + 1) * P], pst[:])

            if c == 0:
                nc.sync.dma_start(Wq_sb[:], Wq_r)
            # ---- q heads ----
            for h in range(NHL):
                psq = pq.tile([P, CS], F32, tag="psq")
                for ko in range(KO):
                    nc.tensor.matmul(
                        psq[:], Wq_sb[:, ko, h * HD:(h + 1) * HD], xt[:, ko, :],
                        start=(ko == 0), stop=(ko == KO - 1))
                # sin is 64-periodic over d, so rot(q)*sin == rot(q*sin):
                # multiply by sin BEFORE the rotation matmul (saves the psum copy)
                qs = stream3.tile([P, CS], BF16, tag="qbf")
                nc.vector.tensor_mul(qs[:], psq[:], sn_t[:])
                psr = pr.tile([P, CS], F32, tag="psr")
                nc.tensor.matmul(psr[:], RT_sb[:], qs[:], start=True, stop=True)
                qro = stream.tile([P, CS], F32, tag="qro")
                nc.vector.tensor_mul(qro[:], psq[:], cs_t[:])
                nc.vector.tensor_add(qro[:], qro[:], psr[:])
                # kappa -> QkT
                mq = stream.tile([P, CS], F32, tag="mq")
                nc.gpsimd.tensor_scalar_min(mq[:], qro[:], 0.0)
                eq = stream.tile([P, CS], F32, tag="eq")
                nc.scalar.activation(eq[:], mq[:], mybir.ActivationFunctionType.Exp)
                nc.vector.scalar_tensor_tensor(
                    QkT[:, h, c * CS:(c + 1) * CS], qro[:], 0.0, eq[:],
                    mybir.AluOpType.max, mybir.AluOpType.add)
                # Qg partial
                nc.vector.tensor_reduce(
                    qg_parts[:, h, c:c + 1], QkT[:, h, c * CS:(c + 1) * CS],
                    mybir.AxisListType.X, mybir.AluOpType.add)

        Wphi_sb = wts.tile([P, KO, NHL * HD], BF16, tag="big")
        nc.sync.dma_start(Wphi_sb[:], Wphi_r)
        Wo_sb = wts.tile([P, NHL, HID], BF16, tag="big")
        nc.sync.dma_start(Wo_sb[:], Wo_r)

        # ================= phase B: Qg, logits, softmax, outer =================
        qg_bf = small.tile([P, NHL], BF16, tag="qgbf")
        qg_f = small.tile([P, NHL], F32, tag="qgf")
        for h in range(NHL):
            nc.vector.tensor_reduce(
                qg_f[:, h:h + 1], qg_parts[:, h, :],
                mybir.AxisListType.X, mybir.AluOpType.add)
        nc.vector.tensor_scalar_mul(qg_bf[:], qg_f[:], 1.0 / S)

        # logits[s] per head: lhsT = KkT tile [d, s-tile], rhs = qg column
        for st in range(NST):
            psl = pmix.tile([P, NHL], F32, tag="mix")
            for h in range(NHL):
                nc.tensor.matmul(
                    psl[:, h:h + 1], KkT[:, st * P:(st + 1) * P],
                    qg_bf[:, h:h + 1], start=True, stop=True)
            nc.vector.tensor_copy(
                logits_sd.rearrange("p h t -> p t h")[:, st, :], psl[:])

        for h in range(NHL):
            lg = logits_sd[:, h, :]                       # [128, 32]
            pmax = small.tile([P, 1], F32, tag="pmax")
            nc.vector.tensor_reduce(pmax[:], lg, mybir.AxisListType.X, mybir.AluOpType.max)
            # global max: transpose pmax -> [1,128], reduce, negate-broadcast back
            pmt = pmix.tile([1, P], F32, tag="mix")
            nc.tensor.transpose(pmt[:], pmax[:], ident_f32[:])
            gmax = small.tile([1, 1], F32, tag="gmax")
            nc.vector.tensor_reduce(gmax[:], pmt[:], mybir.AxisListType.X, mybir.AluOpType.max)
            pngm = pmix.tile([P, 1], F32, tag="mix")
            nc.tensor.matmul(pngm[:], negr_f32[:], gmax[:], start=True, stop=True)
            ngm = small.tile([P, 1], F32, tag="ngm")
            nc.vector.tensor_copy(ngm[:], pngm[:])
            # e = exp(l - gmax), per-partition sums via accum_out
            e_sd = small.tile([P, NST], F32, tag="esd")
            srow = small.tile([P, 1], F32, tag="srow")
            nc.scalar.activation(e_sd[:], lg, mybir.ActivationFunctionType.Exp,
                                 bias=ngm[:], accum_out=srow[:])
            # total = sum_p srow  (fp32 matmul), then rcp broadcast
            ptot = pmix.tile([1, 1], F32, tag="mix")
            nc.tensor.matmul(ptot[:], srow[:], ones_f32[:], start=True, stop=True)
            rcp = small.tile([1, 1], F32, tag="rcp")
            nc.vector.reciprocal(rcp[:], ptot[:])
            prc = pmix.tile([P, 1], F32, tag="mix")
            nc.tensor.matmul(prc[:], onesr_f32[:], rcp[:], start=True, stop=True)
            rcpb = small.tile([P, 1], F32, tag="rcpb")
            nc.vector.tensor_copy(rcpb[:], prc[:])
            nc.vector.tensor_scalar(
                alpha_sd[:, h, :], e_sd[:], rcpb[:], float(S),
                mybir.AluOpType.mult, mybir.AluOpType.mult)

        # outer[h] = sum_st (alpha*Kk_tile)^T... lhsT=KkA [s,d], rhs=v [s,f]
        for h in range(NHL):
            pso = pmix.tile([P, HD], F32, tag="mix")
            for st in range(NST):
                kka = stream3.tile([P, HD], BF16, tag="kka")
                nc.vector.tensor_scalar_mul(
                    kka[:], Kk_sd[:, st, :], alpha_sd[:, h, st:st + 1])
                nc.tensor.matmul(pso[:], kka[:], v_sd[:, st, :],
                                 start=(st == 0), stop=(st == NST - 1))
            nc.vector.tensor_copy(outer_bf[:, h, :], pso[:])

        # ================= phase C: result_attn, ctx, o_proj =================
        for c in range(NCH):
            xt = xp.tile([P, KO, CS], BF16, tag="xt")
            nc.sync.dma_start(xt[:], xT_r[:, :, c * CS:(c + 1) * CS])
            ctx_bf = stream.tile([P, NHL, CS], BF16, tag="ctx")
            for h in range(NHL):
                psp = pq.tile([P, CS], F32, tag="psq")
                for ko in range(KO):
                    nc.tensor.matmul(
                        psp[:], Wphi_sb[:, ko, h * HD:(h + 1) * HD], xt[:, ko, :],
                        start=(ko == 0), stop=(ko == KO - 1))
                phiT = stream.tile([P, CS], F32, tag="phiT")
                nc.scalar.activation(phiT[:], psp[:], mybir.ActivationFunctionType.Identity, bias=bphi_sb[:, h:h + 1])
                psr = pr.tile([P, CS], F32, tag="psr")
                nc.tensor.matmul(psr[:], outer_bf[:, h, :],
                                 QkT[:, h, c * CS:(c + 1) * CS], start=True, stop=True)
                for st in range(4):
                    nc.vector.tensor_mul(
                        ctx_bf[:, h, st * P:(st + 1) * P],
                        phiT[:, st * P:(st + 1) * P], psr[:, st * P:(st + 1) * P])
            # o_proj for the 4 s-tiles of this chunk
            for st in range(4):
                stg = c * 4 + st
                for n in range(4):
                    pso2 = po.tile([P, 512], F32, tag="psout")
                    for h in range(NHL):
                        nc.tensor.matmul(
                            pso2[:], ctx_bf[:, h, st * P:(st + 1) * P],
                            Wo_sb[:, h, n * 512:(n + 1) * 512],
                            start=(h == 0), stop=(h == NHL - 1))
                    ob = stream.tile([P, 512], F32, tag="ob")
                    if (st + n) % 2 == 0:
                        nc.vector.tensor_copy(ob[:], pso2[:])
                    else:
                        nc.scalar.copy(ob[:], pso2[:])
                    nc.sync.dma_start(out_r[:, stg, n * 512:(n + 1) * 512], ob[:])

    nc.compile()
    return nc


def _host_prep(hidden_states, position_ids, Wq, Wk, Wv, Wo, Wphi, bphi):
    B = hidden_states.shape[0]
    # rope tables (match reference fp32 math)
    inv_freq = (1.0 / (ROPE_THETA ** (np.arange(0, HD, 2, dtype=np.float32) / HD))).astype(np.float32)
    in_maps = []
    Rm = np.zeros((P, P), dtype=np.float32)
    Rm[np.arange(64), np.arange(64) + 64] = -1.0
    Rm[np.arange(64) + 64, np.arange(64)] = 1.0
    RT_np = np.ascontiguousarray(Rm.T).astype(BF)
    for b in range(B):
        freqs = position_ids[b].astype(np.float32)[:, None] * inv_freq[None, :]
        emb = np.concatenate([freqs, freqs], axis=1)          # [S, 128]
        cos_b = np.cos(emb).astype(np.float32)
        sin_b = np.sin(emb).astype(np.float32)
        xT_b = np.ascontiguousarray(hidden_states[b].T).astype(BF)
        cosT_b = np.ascontiguousarray(cos_b.T)
        sinT_b = np.ascontiguousarray(sin_b.T)
        for g in range(4):
            sl4 = slice(g * 512, (g + 1) * 512)
            sl1 = slice(g * 128, (g + 1) * 128)
            in_maps.append({
                "xT": xT_b,
                "cosT": cosT_b, "sinT": sinT_b,
                "cos_sd": cos_b, "sin_sd": sin_b,
                "Wq": np.ascontiguousarray(Wq[:, sl4]).astype(BF),
                "Wkv": np.ascontiguousarray(
                    np.concatenate([Wk[:, sl1], Wv[:, sl1]], axis=1)).astype(BF),
                "Wphi": np.ascontiguousarray(Wphi[:, sl4]).astype(BF),
                "Wo": np.ascontiguousarray(Wo[sl4, :]).astype(BF),
                "bphi": np.ascontiguousarray(bphi[sl4]).astype(np.float32),
                "RT": RT_np,
            })
    return in_maps


def kernel(hidden_states, position_ids, Wq, Wk, Wv, Wo, Wphi, bphi, _trace=False):
    if "nc" not in _CACHE:
        _CACHE["nc"] = _build()
    nc = _CACHE["nc"]
    in_maps = _host_prep(np.asarray(hidden_states), np.asarray(position_ids),
                         np.asarray(Wq), np.asarray(Wk), np.asarray(Wv),
                         np.asarray(Wo), np.asarray(Wphi), np.asarray(bphi))
    res = run_bass_kernel_spmd(nc, in_maps, list(range(8)), trace=_trace)
    _CACHE["last_res"] = res
    B = hidden_states.shape[0]
    out = np.empty((B, S, HID), dtype=np.float32)
    for b in range(B):
        acc = res.results[b * 4 + 0]["out"].astype(np.float32)
        for g in range(1, 4):
            acc = acc + res.results[b * 4 + g]["out"]
        out[b] = acc
    return out



# revision 6
# speedup vs baseline: 1.5111x; 1.5111x over previous
"""Trainium2 Bass kernel for LlamaRALAAttention (B=2, S=4096, HID=2048, NH=16, NKV=4, HD=128).

Sharding: 8 cores = DP(batch=2) x TP(kv-head groups=4). Core c handles batch c//4,
kv group c%4 (4 q heads + 1 kv head). o_proj partials summed on host.

fp8 DoubleRow strategy (cost model: DR fp8 = 0.5 cyc/out-col, K=256/instr = 4x bf16):
  q proj:   1-pass fp8 (noise dilutes through Qg-mean and the positive rank-1-ish
            outer contraction; verified in numpy precision sim).
  k/v, phi: 3-pass fp8 (x8@W8 + dx8@W8 + x8@dW8) -> better than bf16 accuracy at
            0.75x bf16 PE cost. dx8/dW8 are fp8 residuals (no extra scaling needed,
            fp8 exponent covers them).
  o proj:   3-pass fp8 with on-chip ctx hi/lo fp8 split.
  Value-path storage fp16 (Kk, v, phi, outer); QkT fp8 (diluted like q).
Scales (powers of 2, folded into tables/drain scales/host):
  x*16, W*64 -> psum q/k = 1024x (rope tables carry 1/1024); v drain 1/1024.
  alpha carries 1/16 (fp16 outer range); phi drain carries 16/2^17; ctx stored
  = ctx/2^17 (fp8 range); out drain *2048 restores.
Layouts: q/phi/result/ctx in [d,s]; k/v in [s,d] (rope on free dim, alpha per-
  partition); KkT via PE transpose for logits matvecs.
"""

import sys

sys.path.insert(0, "/opt/trn_rl_repo")

import numpy as np
import ml_dtypes

import concourse.bass as bass
import concourse.mybir as mybir
import concourse.tile as tile
from concourse import bacc
from concourse.bass_utils import run_bass_kernel_spmd
from concourse.masks import make_identity

P = 128
S = 4096
HID = 2048
HD = 128
NHL = 4            # q heads per core
KO2 = 8            # 2048 / 256 contraction instrs per DR pass
CS = 512           # token chunk
NCH = S // CS      # 8
NST = S // P       # 32
ROPE_THETA = 10000.0

SX = 16.0          # x fp8 scale
SW = 64.0          # weight fp8 scale
SCTX = 131072.0    # ctx stored = ctx/SCTX (2^17)
SAL = 16.0         # alpha folded scale

F32 = mybir.dt.float32
F16 = mybir.dt.float16
BF16 = mybir.dt.bfloat16
F8 = mybir.dt.float8e4
NPF8 = ml_dtypes.float8_e4m3
NPH = np.float16
NPBF = ml_dtypes.bfloat16
DR = mybir.MatmulPerfMode.DoubleRow
AX = mybir.AxisListType.X
OP = mybir.AluOpType
ACT = mybir.ActivationFunctionType

_CACHE = {}


def _build():
    nc = bacc.Bacc("TRN2", target_bir_lowering=False, debug=False, num_devices=8)

    x8 = nc.dram_tensor("x8", [P, KO2, 2, S], F8, kind="ExternalInput").ap()
    dx8 = nc.dram_tensor("dx8", [P, KO2, 2, S], F8, kind="ExternalInput").ap()
    Wq8 = nc.dram_tensor("Wq8", [P, KO2, 2, NHL * HD], F8, kind="ExternalInput").ap()
    Wkv8 = nc.dram_tensor("Wkv8", [P, KO2, 2, 2 * HD], F8, kind="ExternalInput").ap()
    dWkv8 = nc.dram_tensor("dWkv8", [P, KO2, 2, 2 * HD], F8, kind="ExternalInput").ap()
    Wphi8 = nc.dram_tensor("Wphi8", [P, KO2, 2, NHL * HD], F8, kind="ExternalInput").ap()
    dWphi8 = nc.dram_tensor("dWphi8", [P, KO2, 2, NHL * HD], F8, kind="ExternalInput").ap()
    WoH8 = nc.dram_tensor("WoH8", [P, 2, 2, HID], F8, kind="ExternalInput").ap()
    WoL8 = nc.dram_tensor("WoL8", [P, 2, 2, HID], F8, kind="ExternalInput").ap()
    cosqT = nc.dram_tensor("cosqT", [P, S], F16, kind="ExternalInput").ap()
    sinqT = nc.dram_tensor("sinqT", [P, S], F16, kind="ExternalInput").ap()
    kcs = nc.dram_tensor("kcs", [P, NST, 2, HD], F16, kind="ExternalInput").ap()
    bphi_s = nc.dram_tensor("bphi_s", [P, NHL], F32, kind="ExternalInput").ap()
    RT = nc.dram_tensor("RT", [P, P], F16, kind="ExternalInput").ap()
    out = nc.dram_tensor("out", [P, NST, HID], BF16, kind="ExternalOutput").ap()

    from contextlib import ExitStack
    with tile.TileContext(nc) as tc, ExitStack() as es:
        res = es.enter_context(tc.tile_pool(name="res", bufs=1))
        wts = es.enter_context(tc.tile_pool(name="wts", bufs=1))
        xp = es.enter_context(tc.tile_pool(name="xp", bufs=2))
        dxp = es.enter_context(tc.tile_pool(name="dxp", bufs=2))
        tb = es.enter_context(tc.tile_pool(name="tb", bufs=2))
        st3 = es.enter_context(tc.tile_pool(name="st3", bufs=3))
        ctxp = es.enter_context(tc.tile_pool(name="ctxp", bufs=2))
        outp = es.enter_context(tc.tile_pool(name="outp", bufs=4))
        small = es.enter_context(tc.tile_pool(name="small", bufs=4))
        pq = es.enter_context(tc.tile_pool(name="pq", bufs=2, space="PSUM"))
        pr = es.enter_context(tc.tile_pool(name="pr", bufs=2, space="PSUM"))
        pphi = es.enter_context(tc.tile_pool(name="pphi", bufs=2, space="PSUM"))
        pmix = es.enter_context(tc.tile_pool(name="pmix", bufs=2, space="PSUM"))

        # ---- weights / tables ----
        Wq_sb = wts.tile([P, KO2, 2, NHL * HD], F8)
        nc.sync.dma_start(Wq_sb[:], Wq8)
        Wkv_sb = wts.tile([P, KO2, 2, 2 * HD], F8)
        nc.sync.dma_start(Wkv_sb[:], Wkv8)
        dWkv_sb = wts.tile([P, KO2, 2, 2 * HD], F8)
        nc.sync.dma_start(dWkv_sb[:], dWkv8)
        Wphi_sb = wts.tile([P, KO2, 2, NHL * HD], F8)
        nc.sync.dma_start(Wphi_sb[:], Wphi8)
        dWphi_sb = wts.tile([P, KO2, 2, NHL * HD], F8)
        nc.sync.dma_start(dWphi_sb[:], dWphi8)
        WoH_sb = wts.tile([P, 2, 2, HID], F8)
        nc.sync.dma_start(WoH_sb[:], WoH8)
        WoL_sb = wts.tile([P, 2, 2, HID], F8)
        nc.sync.dma_start(WoL_sb[:], WoL8)
        RT_sb = res.tile([P, P], F16)
        nc.sync.dma_start(RT_sb[:], RT)
        bphi_sb = res.tile([P, NHL], F32)
        nc.sync.dma_start(bphi_sb[:], bphi_s)

        id16 = res.tile([P, P], F16)
        make_identity(nc, id16[:])
        idf32 = res.tile([P, P], F32)
        make_identity(nc, idf32[:])
        ones_f32 = res.tile([P, 1], F32)
        nc.vector.memset(ones_f32[:], 1.0)
        onesr_f32 = res.tile([1, P], F32)
        nc.vector.memset(onesr_f32[:], 1.0)
        negr_f32 = res.tile([1, P], F32)
        nc.vector.memset(negr_f32[:], -1.0)

        # ---- residents ----
        QkT8 = res.tile([P, NHL, S], F8)          # kappa(rope(q)), [d,s], fp8
        phiT16 = res.tile([P, NHL, S], F16)       # phi*SAL/SCTX, [d,s]
        Kk16 = res.tile([P, NST, HD], F16)        # kappa(rope(k)), [s,d]
        KkT16 = res.tile([P, S], F16)             # [d,s]
        v16 = res.tile([P, NST, HD], F16)         # [s,d]
        outer16 = res.tile([P, NHL, HD], F16)     # outer/SAL, [d,f]
        qg_parts = res.tile([P, NHL, NCH], F32)
        logits_sd = res.tile([P, NST, NHL], F32)
        alpha_sd = res.tile([P, NHL, NST], F32)   # alpha/SAL

        # ================= phase A: q/k/v/phi projections + rope + kappa =================
        for c in range(NCH):
            sl = slice(c * CS, (c + 1) * CS)
            xt = xp.tile([P, KO2, 2, CS], F8, tag="x")
            nc.sync.dma_start(xt[:], x8[:, :, :, sl])
            dxt = dxp.tile([P, KO2, 2, CS], F8, tag="dx")
            nc.sync.dma_start(dxt[:], dx8[:, :, :, sl])
            cq = tb.tile([P, CS], F16, tag="cq")
            nc.sync.dma_start(cq[:], cosqT[:, sl])
            sq = tb.tile([P, CS], F16, tag="sq")
            nc.sync.dma_start(sq[:], sinqT[:, sl])
            kct = tb.tile([P, 4, 2, HD], F16, tag="kc")
            nc.sync.dma_start(kct[:], kcs[:, c * 4:(c + 1) * 4, :, :])

            # ---- k/v (3-pass fp8 DR), [s,d] ----
            for st in range(4):
                stg = c * 4 + st
                ssl = slice(st * P, (st + 1) * P)
                pskv = pmix.tile([P, 2 * HD], F32, tag="mix")
                passes = [(xt, Wkv_sb), (dxt, Wkv_sb), (xt, dWkv_sb)]
                n = 0
                for lt, rt in passes:
                    for ko in range(KO2):
                        nc.tensor.matmul(
                            pskv[:], lt[:, ko, :, ssl], rt[:, ko, :, :],
                            start=(n == 0), stop=(n == 3 * KO2 - 1), perf_mode=DR)
                        n += 1
                k16 = st3.tile([P, HD], F16, tag="k16")
                nc.scalar.activation(k16[:], pskv[:, :HD], ACT.Identity)
                nc.scalar.activation(v16[:, stg, :], pskv[:, HD:], ACT.Identity,
                                     scale=1.0 / (SX * SW))
                # rope-k on free dim halves (tables carry 1/1024)
                kr = st3.tile([P, HD], F16, tag="kr")
                nc.vector.tensor_mul(kr[:], k16[:], kct[:, st, 0, :])
                t2 = st3.tile([P, 64], F16, tag="t2")
                nc.vector.tensor_mul(t2[:], k16[:, 64:], kct[:, st, 1, :64])
                nc.vector.tensor_sub(kr[:, :64], kr[:, :64], t2[:])
                t3 = st3.tile([P, 64], F16, tag="t3")
                nc.vector.tensor_mul(t3[:], k16[:, :64], kct[:, st, 1, 64:])
                nc.vector.tensor_add(kr[:, 64:], kr[:, 64:], t3[:])
                # kappa = max(x,0) + min(exp(x),1)
                ek = st3.tile([P, HD], F16, tag="ek")
                nc.scalar.activation(ek[:], kr[:], ACT.Exp)
                tk = st3.tile([P, HD], F16, tag="tk")
                nc.gpsimd.tensor_scalar_min(tk[:], ek[:], 1.0)
                nc.vector.scalar_tensor_tensor(
                    Kk16[:, stg, :], kr[:], 0.0, tk[:], OP.max, OP.add)
                pst = pmix.tile([P, P], F16, tag="mix")
                nc.tensor.transpose(pst[:], Kk16[:, stg, :], id16[:])
                nc.vector.tensor_copy(KkT16[:, stg * P:(stg + 1) * P], pst[:])

            # ---- q (1-pass fp8 DR) + phi (3-pass), [d,s] ----
            for h in range(NHL):
                hsl = slice(h * HD, (h + 1) * HD)
                psq = pq.tile([P, CS], F32, tag="q")
                for n2 in range(2):
                    nsl = slice(n2 * 256, (n2 + 1) * 256)
                    for ko in range(KO2):
                        nc.tensor.matmul(
                            psq[:, nsl], Wq_sb[:, ko, :, hsl], xt[:, ko, :, nsl],
                            start=(ko == 0), stop=(ko == KO2 - 1), perf_mode=DR)
                q16 = st3.tile([P, CS], F16, tag="q16")
                nc.scalar.activation(q16[:], psq[:], ACT.Identity)
                qs = st3.tile([P, CS], F16, tag="qs")
                nc.vector.tensor_mul(qs[:], q16[:], sq[:])
                qro = st3.tile([P, CS], F16, tag="qro")
                nc.vector.tensor_mul(qro[:], q16[:], cq[:])
                psr = pr.tile([P, CS], F32, tag="r")
                nc.tensor.matmul(psr[:], RT_sb[:], qs[:], start=True, stop=False)
                nc.tensor.matmul(psr[:], id16[:], qro[:], start=False, stop=True)
                ea = st3.tile([P, CS], F16, tag="ea")
                nc.scalar.activation(ea[:], psr[:], ACT.Exp)
                tq = st3.tile([P, CS], F16, tag="tq")
                nc.vector.tensor_scalar_min(tq[:], ea[:], 1.0)
                nc.vector.scalar_tensor_tensor(
                    QkT8[:, h, sl], psr[:], 0.0, tq[:], OP.max, OP.add)
                nc.vector.tensor_reduce(
                    qg_parts[:, h, c:c + 1], QkT8[:, h, sl], AX, OP.add)
                # phi
                psp = pphi.tile([P, CS], F32, tag="p")
                passes = [(xt, Wphi_sb), (dxt, Wphi_sb), (xt, dWphi_sb)]
                for n2 in range(2):
                    nsl = slice(n2 * 256, (n2 + 1) * 256)
                    n = 0
                    for lt, rt in passes:
                        for ko in range(KO2):
                            nc.tensor.matmul(
                                psp[:, nsl], rt[:, ko, :, hsl], lt[:, ko, :, nsl],
                                start=(n == 0), stop=(n == 3 * KO2 - 1), perf_mode=DR)
                            n += 1
                nc.scalar.activation(phiT16[:, h, sl], psp[:], ACT.Identity,
                                     bias=bphi_sb[:, h:h + 1],
                                     scale=SAL / (SX * SW * SCTX))

        # ================= phase B: Qg, logits, softmax, outer =================
        qg_f = small.tile([P, NHL], F32, tag="qgf")
        for h in range(NHL):
            nc.vector.tensor_reduce(qg_f[:, h:h + 1], qg_parts[:, h, :], AX, OP.add)
        qg16 = small.tile([P, NHL], F16, tag="qg16")
        nc.vector.tensor_scalar_mul(qg16[:], qg_f[:], 1.0 / S)

        psl = pr.tile([P, NST, NHL], F32, tag="r")
        for st in range(NST):
            nc.tensor.matmul(psl[:, st, :], KkT16[:, st * P:(st + 1) * P],
                             qg16[:], start=True, stop=True)
        nc.vector.tensor_copy(logits_sd[:], psl[:])

        for h in range(NHL):
            lg = logits_sd[:, :, h]                       # [128, 32] stride NHL
            pmax = small.tile([P, 1], F32, tag="pmax")
            nc.vector.tensor_reduce(pmax[:], lg, AX, OP.max)
            pmt = pmix.tile([1, P], F32, tag="mix")
            nc.tensor.transpose(pmt[:], pmax[:], idf32[:])
            gmax = small.tile([1, 1], F32, tag="gmax")
            nc.vector.tensor_reduce(gmax[:], pmt[:], AX, OP.max)
            pngm = pmix.tile([P, 1], F32, tag="mix")
            nc.tensor.matmul(pngm[:], negr_f32[:], gmax[:], start=True, stop=True)
            ngm = small.tile([P, 1], F32, tag="ngm")
            nc.vector.tensor_copy(ngm[:], pngm[:])
            e_sd = small.tile([P, NST], F32, tag="esd")
            srow = small.tile([P, 1], F32, tag="srow")
            nc.scalar.activation(e_sd[:], lg, ACT.Exp, bias=ngm[:], accum_out=srow[:])
            ptot = pmix.tile([1, 1], F32, tag="mix")
            nc.tensor.matmul(ptot[:], srow[:], ones_f32[:], start=True, stop=True)
            rcp = small.tile([1, 1], F32, tag="rcp")
            nc.vector.reciprocal(rcp[:], ptot[:])
            prc = pmix.tile([P, 1], F32, tag="mix")
            nc.tensor.matmul(prc[:], onesr_f32[:], rcp[:], start=True, stop=True)
            rcpb = small.tile([P, 1], F32, tag="rcpb")
            nc.vector.tensor_copy(rcpb[:], prc[:])
            nc.vector.tensor_scalar(
                alpha_sd[:, h, :], e_sd[:], rcpb[:], float(S) / SAL,
                OP.mult, OP.mult)

        for h in range(NHL):
            pso = pq.tile([P, HD], F32, tag="q")
            for st in range(NST):
                kka = st3.tile([P, HD], F16, tag="kka")
                nc.vector.tensor_scalar_mul(
                    kka[:], Kk16[:, st, :], alpha_sd[:, h, st:st + 1])
                nc.tensor.matmul(pso[:], kka[:], v16[:, st, :],
                                 start=(st == 0), stop=(st == NST - 1))
            nc.scalar.activation(outer16[:, h, :], pso[:], ACT.Identity)

        # ================= phase C: result, ctx hi/lo, o_proj =================
        dr_engine = 0
        for c in range(NCH):
            sl = slice(c * CS, (c + 1) * CS)
            ctxh = ctxp.tile([P, NHL, CS], F8, tag="ch")
            ctxl = ctxp.tile([P, NHL, CS], F8, tag="cl")
            for h in range(NHL):
                psr = pr.tile([P, CS], F32, tag="r")
                nc.tensor.matmul(psr[:], outer16[:, h, :], QkT8[:, h, sl],
                                 start=True, stop=True)
                cx = st3.tile([P, CS], F16, tag="cx")
                nc.vector.tensor_mul(cx[:], phiT16[:, h, sl], psr[:])
                nc.scalar.activation(ctxh[:, h, :], cx[:], ACT.Identity)
                nc.vector.scalar_tensor_tensor(
                    ctxl[:, h, :], ctxh[:, h, :], -1.0, cx[:], OP.mult, OP.add)
            for st in range(4):
                stg = c * 4 + st
                ssl = slice(st * P, (st + 1) * P)
                for nq in range(4):
                    if nq % 2 == 0:
                        po = pq.tile([P, CS], F32, tag="q")
                    else:
                        po = pphi.tile([P, CS], F32, tag="p")
                    passes = [(ctxh, WoH_sb), (ctxl, WoH_sb), (ctxh, WoL_sb)]
                    for n2 in range(2):
                        n = 0
                        for ct, wt in passes:
                            for hp in range(2):
                                nc.tensor.matmul(
                                    po[:, n2 * 256:(n2 + 1) * 256],
                                    ct[:, 2 * hp:2 * hp + 2, ssl],
                                    wt[:, hp, :, nq * CS + n2 * 256:nq * CS + (n2 + 1) * 256],
                                    start=(n == 0), stop=(n == 5), perf_mode=DR)
                                n += 1
                    ob = outp.tile([P, CS], BF16, tag="ob")
                    eng = dr_engine % 2
                    dr_engine += 1
                    if eng == 0:
                        nc.vector.tensor_scalar_mul(ob[:], po[:], SCTX / SW)
                    else:
                        nc.scalar.activation(ob[:], po[:], ACT.Identity, scale=SCTX / SW)
                    nc.sync.dma_start(out[:, stg, nq * CS:(nq + 1) * CS], ob[:])

    nc.compile()
    return nc


def _host_prep(hidden_states, position_ids, Wq, Wk, Wv, Wo, Wphi, bphi):
    B = hidden_states.shape[0]

    def q8(a):
        return np.clip(a, -240, 240).astype(NPF8)

    def split8(a):  # fp8 hi + residual
        hi = q8(a)
        lo = q8(a - hi.astype(np.float32))
        return hi, lo

    def wlay(W, sc=True):  # [2048, M] -> [p, ko, 2, M]
        Wl = (W * SW).astype(np.float32) if sc else W
        return np.ascontiguousarray(
            Wl.reshape(KO2, 2, P, -1).transpose(2, 0, 1, 3))

    inv_freq = (1.0 / (ROPE_THETA ** (np.arange(0, HD, 2, dtype=np.float32) / HD))).astype(np.float32)
    Rm = np.zeros((P, P), dtype=np.float32)
    Rm[np.arange(64), np.arange(64) + 64] = -1.0
    Rm[np.arange(64) + 64, np.arange(64)] = 1.0
    RT_np = np.ascontiguousarray(Rm.T).astype(NPH)

    in_maps = []
    for b in range(B):
        freqs = position_ids[b].astype(np.float32)[:, None] * inv_freq[None, :]
        emb = np.concatenate([freqs, freqs], axis=1)          # [S, 128]
        cos_b = np.cos(emb) / (SX * SW)
        sin_b = np.sin(emb) / (SX * SW)
        cosqT_b = np.ascontiguousarray(cos_b.T).astype(NPH)
        sinqT_b = np.ascontiguousarray(sin_b.T).astype(NPH)
        # kcs[p, st, 0/1, d]
        kcs_b = np.ascontiguousarray(
            np.stack([cos_b.reshape(NST, P, HD), sin_b.reshape(NST, P, HD)],
                     axis=2).transpose(1, 0, 2, 3)).astype(NPH)
        xs = (hidden_states[b].T * SX).astype(np.float32)      # [HID, S]
        x8_full = q8(xs)
        dx8_full = q8(xs - x8_full.astype(np.float32))
        x8_b = np.ascontiguousarray(
            x8_full.reshape(KO2, 2, P, S).transpose(2, 0, 1, 3))
        dx8_b = np.ascontiguousarray(
            dx8_full.reshape(KO2, 2, P, S).transpose(2, 0, 1, 3))
        for g in range(4):
            sl4 = slice(g * 512, (g + 1) * 512)
            sl1 = slice(g * 128, (g + 1) * 128)
            Wq_l = q8(wlay(Wq[:, sl4]))
            Wkv_hi, Wkv_lo = split8(wlay(np.concatenate([Wk[:, sl1], Wv[:, sl1]], axis=1)))
            Wphi_hi, Wphi_lo = split8(wlay(Wphi[:, sl4]))
            # Wo [512, 2048] -> [p, hp, 2, n]
            Wo_l = (Wo[sl4, :] * SW).astype(np.float32).reshape(2, 2, P, HID).transpose(2, 0, 1, 3)
            Wo_hi, Wo_lo = split8(np.ascontiguousarray(Wo_l))
            in_maps.append({
                "x8": x8_b, "dx8": dx8_b,
                "Wq8": Wq_l, "Wkv8": Wkv_hi, "dWkv8": Wkv_lo,
                "Wphi8": Wphi_hi, "dWphi8": Wphi_lo,
                "WoH8": Wo_hi, "WoL8": Wo_lo,
                "cosqT": cosqT_b, "sinqT": sinqT_b, "kcs": kcs_b,
                "bphi_s": np.ascontiguousarray(
                    (bphi[sl4] * SAL / SCTX).astype(np.float32).reshape(NHL, P).T),
                "RT": RT_np,
            })
    return in_maps


def kernel(hidden_states, position_ids, Wq, Wk, Wv, Wo, Wphi, bphi, _trace=False):
    if "nc" not in _CACHE:
        _CACHE["nc"] = _build()
    nc = _CACHE["nc"]
    in_maps = _host_prep(np.asarray(hidden_states), np.asarray(position_ids),
                         np.asarray(Wq), np.asarray(Wk), np.asarray(Wv),
                         np.asarray(Wo), np.asarray(Wphi), np.asarray(bphi))
    res = run_bass_kernel_spmd(nc, in_maps, list(range(8)), trace=_trace)
    _CACHE["last_res"] = res
    B = hidden_states.shape[0]
    out = np.empty((B, S, HID), dtype=np.float32)
    for b in range(B):
        acc = res.results[b * 4 + 0]["out"].astype(np.float32)
        for g in range(1, 4):
            acc = acc + res.results[b * 4 + g]["out"].astype(np.float32)
        out[b] = acc.reshape(P, NST, HID).transpose(1, 0, 2).reshape(S, HID)
    return out


# revision 29
# speedup vs baseline: 1.6967x; 1.1228x over previous
"""Trainium2 Bass kernel for LlamaRALAAttention (B=2, S=4096, HID=2048, NH=16, NKV=4, HD=128).

Sharding: 8 cores = DP(batch=2) x TP(kv-head groups=4). Core c handles batch c//4,
kv group c%4 (4 q heads + 1 kv head). o_proj partials summed on host.

fp8 DoubleRow strategy (cost model: DR fp8 = 0.5 cyc/out-col, K=256/instr = 4x bf16):
  q proj:   1-pass fp8 (noise dilutes through Qg-mean and the positive rank-1-ish
            outer contraction; verified in numpy precision sim).
  k/v, phi: 3-pass fp8 (x8@W8 + dx8@W8 + x8@dW8) -> better than bf16 accuracy at
            0.75x bf16 PE cost. dx8/dW8 are fp8 residuals (no extra scaling needed,
            fp8 exponent covers them).
  o proj:   3-pass fp8 with on-chip ctx hi/lo fp8 split.
  Value-path storage fp16 (Kk, v, phi, outer); QkT fp8 (diluted like q).
Scales (powers of 2, folded into tables/drain scales/host):
  x*16, W*64 -> psum q/k = 1024x (rope tables carry 1/1024); v drain 1/1024.
  alpha carries 1/16 (fp16 outer range); phi drain carries 16/2^17; ctx stored
  = ctx/2^17 (fp8 range); out drain *2048 restores.
Layouts: q/phi/result/ctx in [d,s]; k/v in [s,d] (rope on free dim, alpha per-
  partition); KkT via PE transpose for logits matvecs.
"""

import sys

sys.path.insert(0, "/opt/trn_rl_repo")

import numpy as np
import ml_dtypes

import concourse.bass as bass
import concourse.mybir as mybir
import concourse.tile as tile
from concourse import bacc
from concourse.bass_utils import run_bass_kernel_spmd
from concourse.masks import make_identity

P = 128
S = 4096
HID = 2048
HD = 128
NHL = 4            # q heads per core
KO2 = 8            # 2048 / 256 contraction instrs per DR pass
CS = 512           # token chunk
NCH = S // CS      # 8
NST = S // P       # 32
ROPE_THETA = 10000.0

SX = 16.0          # x fp8 scale
SW = 64.0          # weight fp8 scale
SCTX = 131072.0    # ctx stored = ctx/SCTX (2^17)
SAL = 16.0         # alpha folded scale

F32 = mybir.dt.float32
F16 = mybir.dt.float16
BF16 = mybir.dt.bfloat16
F8 = mybir.dt.float8e4
NPF8 = ml_dtypes.float8_e4m3
NPH = np.float16
NPBF = ml_dtypes.bfloat16
DR = mybir.MatmulPerfMode.DoubleRow
AX = mybir.AxisListType.X
OP = mybir.AluOpType
ACT = mybir.ActivationFunctionType

_CACHE = {}


def _build():
    nc = bacc.Bacc("TRN2", target_bir_lowering=False, debug=False, num_devices=8)

    x8 = nc.dram_tensor("x8", [P, KO2, 2, S], F8, kind="ExternalInput").ap()
    dx8 = nc.dram_tensor("dx8", [P, KO2, 2, S], F8, kind="ExternalInput").ap()
    Wq8 = nc.dram_tensor("Wq8", [P, KO2, 2, NHL * HD], F8, kind="ExternalInput").ap()
    Wkv8 = nc.dram_tensor("Wkv8", [P, KO2, 2, 2 * HD], F8, kind="ExternalInput").ap()
    dWkv8 = nc.dram_tensor("dWkv8", [P, KO2, 2, 2 * HD], F8, kind="ExternalInput").ap()
    Wphi8 = nc.dram_tensor("Wphi8", [P, KO2, 2, NHL * HD], F8, kind="ExternalInput").ap()
    dWphi8 = nc.dram_tensor("dWphi8", [P, KO2, 2, NHL * HD], F8, kind="ExternalInput").ap()
    WoH8 = nc.dram_tensor("WoH8", [P, 2, 2, HID], F8, kind="ExternalInput").ap()
    WoL8 = nc.dram_tensor("WoL8", [P, 2, 2, HID], F8, kind="ExternalInput").ap()
    cosqT = nc.dram_tensor("cosqT", [P, S], F16, kind="ExternalInput").ap()
    sinqT = nc.dram_tensor("sinqT", [P, S], F16, kind="ExternalInput").ap()
    kcs = nc.dram_tensor("kcs", [P, NST, 2, HD], F16, kind="ExternalInput").ap()
    bphi_s = nc.dram_tensor("bphi_s", [P, NHL], F32, kind="ExternalInput").ap()
    RT = nc.dram_tensor("RT", [P, P], F16, kind="ExternalInput").ap()
    out = nc.dram_tensor("out", [P, NST, HID], BF16, kind="ExternalOutput").ap()

    from contextlib import ExitStack
    with tile.TileContext(nc) as tc, ExitStack() as es:
        res = es.enter_context(tc.tile_pool(name="res", bufs=1))
        wts = es.enter_context(tc.tile_pool(name="wts", bufs=1))
        xp = es.enter_context(tc.tile_pool(name="xp", bufs=2))
        dxp = es.enter_context(tc.tile_pool(name="dxp", bufs=2))
        tb = es.enter_context(tc.tile_pool(name="tb", bufs=2))
        st3 = es.enter_context(tc.tile_pool(name="st3", bufs=3))
        ctxp = es.enter_context(tc.tile_pool(name="ctxp", bufs=2))
        outp = es.enter_context(tc.tile_pool(name="outp", bufs=2))
        small = es.enter_context(tc.tile_pool(name="small", bufs=4))
        pq = es.enter_context(tc.tile_pool(name="pq", bufs=2, space="PSUM"))
        pr = es.enter_context(tc.tile_pool(name="pr", bufs=2, space="PSUM"))
        pphi = es.enter_context(tc.tile_pool(name="pphi", bufs=2, space="PSUM"))
        pmix = es.enter_context(tc.tile_pool(name="pmix", bufs=2, space="PSUM"))

        # ---- weights / tables (kv first; the rest stream in during chunk 0) ----
        Wkv_sb = wts.tile([P, KO2, 2, 2 * HD], F8)
        nc.sync.dma_start(Wkv_sb[:], Wkv8)
        dWkv_sb = wts.tile([P, KO2, 2, 2 * HD], F8)
        nc.sync.dma_start(dWkv_sb[:], dWkv8)
        RT_sb = res.tile([P, P], F16)
        nc.sync.dma_start(RT_sb[:], RT)
        bphi_sb = res.tile([P, NHL], F32)
        nc.sync.dma_start(bphi_sb[:], bphi_s)
        Wq_sb = wts.tile([P, KO2, 2, NHL * HD], F8)
        Wphi_sb = wts.tile([P, KO2, 2, NHL * HD], F8)
        dWphi_sb = wts.tile([P, KO2, 2, NHL * HD], F8)
        WoH_sb = wts.tile([P, 2, 2, HID], F8)
        WoL_sb = wts.tile([P, 2, 2, HID], F8)

        id16 = res.tile([P, P], F16)
        make_identity(nc, id16[:])
        idf32 = res.tile([P, P], F32)
        make_identity(nc, idf32[:])
        ones_f32 = res.tile([P, 1], F32)
        nc.vector.memset(ones_f32[:], 1.0)
        onesr_f32 = res.tile([1, P], F32)
        nc.vector.memset(onesr_f32[:], 1.0)
        negr_f32 = res.tile([1, P], F32)
        nc.vector.memset(negr_f32[:], -1.0)

        # ---- residents ----
        QkT8 = res.tile([P, NHL, S], F8)          # kappa(rope(q)), [d,s], fp8
        phiT16 = res.tile([P, NHL, S], F16)       # phi*SAL/SCTX, [d,s]
        Kk16 = res.tile([P, NST, HD], F16)        # kappa(rope(k)), [s,d]
        KkT16 = res.tile([P, S], F16)             # [d,s]
        v16 = res.tile([P, NST, HD], F16)         # [s,d]
        outer16 = res.tile([P, NHL, HD], F16)     # outer/SAL, [d,f]
        qg_parts = res.tile([P, NHL, NCH], F32)
        logits_sd = res.tile([P, NST, NHL], F32)
        alpha_sd = res.tile([P, NHL, NST], F32)   # alpha/SAL

        # ================= phase A: q/k/v/phi projections + rope + kappa =================
        for c in range(NCH):
            sl = slice(c * CS, (c + 1) * CS)
            xt = xp.tile([P, KO2, 2, CS], F8, tag="x")
            dxt = dxp.tile([P, KO2, 2, CS], F8, tag="dx")
            if c == 0:
                # startup: split x loads so the first kv matmuls start sooner
                nc.sync.dma_start(xt[:, :4], x8[:, :4, :, sl])
                nc.sync.dma_start(xt[:, 4:], x8[:, 4:, :, sl])
                nc.sync.dma_start(dxt[:, :4], dx8[:, :4, :, sl])
                nc.sync.dma_start(dxt[:, 4:], dx8[:, 4:, :, sl])
            else:
                nc.sync.dma_start(xt[:], x8[:, :, :, sl])
                nc.sync.dma_start(dxt[:], dx8[:, :, :, sl])
            cq = tb.tile([P, CS], F16, tag="cq")
            nc.sync.dma_start(cq[:], cosqT[:, sl])
            sq = tb.tile([P, CS], F16, tag="sq")
            nc.sync.dma_start(sq[:], sinqT[:, sl])
            kct = tb.tile([P, 4, 2, HD], F16, tag="kc")
            nc.sync.dma_start(kct[:], kcs[:, c * 4:(c + 1) * 4, :, :])
            if c == 0:
                nc.sync.dma_start(Wq_sb[:], Wq8)
                nc.sync.dma_start(Wphi_sb[:], Wphi8)
                nc.sync.dma_start(dWphi_sb[:], dWphi8)

            # ---- k/v (3-pass fp8 DR), [s,d] ----
            for st in range(4):
                stg = c * 4 + st
                ssl = slice(st * P, (st + 1) * P)
                pskv = pmix.tile([P, 2 * HD], F32, tag="mix")
                passes = [(xt, Wkv_sb), (xt, dWkv_sb), (dxt, Wkv_sb)]
                n = 0
                for lt, rt in passes:
                    for ko in range(KO2):
                        nc.tensor.matmul(
                            pskv[:], lt[:, ko, :, ssl], rt[:, ko, :, :],
                            start=(n == 0), stop=(n == 3 * KO2 - 1), perf_mode=DR)
                        n += 1
                k16 = st3.tile([P, HD], F16, tag="k16")
                nc.scalar.activation(k16[:], pskv[:, :HD], ACT.Identity)
                nc.scalar.activation(v16[:, stg, :], pskv[:, HD:], ACT.Identity,
                                     scale=1.0 / (SX * SW))
                # rope-k on free dim halves (tables carry 1/1024)
                kr = st3.tile([P, HD], F16, tag="kr")
                nc.vector.tensor_mul(kr[:], k16[:], kct[:, st, 0, :])
                t2 = st3.tile([P, 64], F16, tag="t2")
                nc.vector.tensor_mul(t2[:], k16[:, 64:], kct[:, st, 1, :64])
                nc.vector.tensor_sub(kr[:, :64], kr[:, :64], t2[:])
                t3 = st3.tile([P, 64], F16, tag="t3")
                nc.vector.tensor_mul(t3[:], k16[:, :64], kct[:, st, 1, 64:])
                nc.vector.tensor_add(kr[:, 64:], kr[:, 64:], t3[:])
                # kappa = max(x,0) + min(exp(x),1)
                ek = st3.tile([P, HD], F16, tag="ek")
                nc.scalar.activation(ek[:], kr[:], ACT.Exp)
                tk = st3.tile([P, HD], F16, tag="tk")
                nc.gpsimd.tensor_scalar_min(tk[:], ek[:], 1.0)
                nc.vector.scalar_tensor_tensor(
                    Kk16[:, stg, :], kr[:], 0.0, tk[:], OP.max, OP.add)

            # ---- q (1-pass fp8 DR) + phi (3-pass), [d,s] ----
            # Emission order software-pipelines PE: q(h) -> [phi(h-1)] -> rope(h-1)
            # so the Act/DVE chain after each q-proj never stalls the PE.
            def q_proj(h):
                hsl = slice(h * HD, (h + 1) * HD)
                psq = pq.tile([P, CS], F32, tag="q", name=f"psq{h}")
                for n2 in range(2):
                    nsl = slice(n2 * 256, (n2 + 1) * 256)
                    for ko in range(KO2):
                        nc.tensor.matmul(
                            psq[:, nsl], Wq_sb[:, ko, :, hsl], xt[:, ko, :, nsl],
                            start=(ko == 0), stop=(ko == KO2 - 1), perf_mode=DR)
                q16 = st3.tile([P, CS], F16, tag="q16", name=f"q16_{h}")
                nc.scalar.activation(q16[:], psq[:], ACT.Identity)
                qs = st3.tile([P, CS], F16, tag="qs", name=f"qs{h}")
                nc.vector.tensor_mul(qs[:], q16[:], sq[:])
                qro = st3.tile([P, CS], F16, tag="qro", name=f"qro{h}")
                nc.vector.tensor_mul(qro[:], q16[:], cq[:])
                return qs, qro

            def q_rope(h, qs, qro):
                psr = pr.tile([P, CS], F32, tag="r", name=f"psr{h}")
                nc.tensor.matmul(psr[:], RT_sb[:], qs[:], start=True, stop=True)
                xr = st3.tile([P, CS], F16, tag="xr", name=f"xr{h}")
                nc.vector.tensor_add(xr[:], qro[:], psr[:])
                ea = st3.tile([P, CS], F16, tag="ea", name=f"ea{h}")
                nc.scalar.activation(ea[:], xr[:], ACT.Exp)
                tq = st3.tile([P, CS], F16, tag="tq", name=f"tq{h}")
                nc.vector.tensor_scalar_min(tq[:], ea[:], 1.0)
                nc.vector.scalar_tensor_tensor(
                    QkT8[:, h, sl], xr[:], 0.0, tq[:], OP.max, OP.add)
                nc.vector.tensor_reduce(
                    qg_parts[:, h, c:c + 1], QkT8[:, h, sl], AX, OP.add)

            def phi_proj(h):
                hsl = slice(h * HD, (h + 1) * HD)
                psp = pphi.tile([P, CS], F32, tag="p", name=f"psp{h}")
                passes = [(xt, Wphi_sb), (xt, dWphi_sb), (dxt, Wphi_sb)]
                for n2 in range(2):
                    nsl = slice(n2 * 256, (n2 + 1) * 256)
                    n = 0
                    for lt, rt in passes:
                        for ko in range(KO2):
                            nc.tensor.matmul(
                                psp[:, nsl], rt[:, ko, :, hsl], lt[:, ko, :, nsl],
                                start=(n == 0), stop=(n == 3 * KO2 - 1), perf_mode=DR)
                            n += 1
                nc.scalar.activation(phiT16[:, h, sl], psp[:], ACT.Identity,
                                     bias=bphi_sb[:, h:h + 1],
                                     scale=SAL / (SX * SW * SCTX))

            qp0 = q_proj(0)
            # KkT transposes (PE) here: Kk16 for early s-tiles is ready by now
            for st in range(4):
                stg = c * 4 + st
                pst = pr.tile([P, P], F16, tag="r", name=f"pst{st}")
                nc.tensor.transpose(pst[:], Kk16[:, stg, :], id16[:])
                nc.vector.tensor_copy(KkT16[:, stg * P:(stg + 1) * P], pst[:])
            qp1 = q_proj(1)
            phi_proj(0)
            q_rope(0, *qp0)
            qp2 = q_proj(2)
            phi_proj(1)
            q_rope(1, *qp1)
            qp3 = q_proj(3)
            phi_proj(2)
            q_rope(2, *qp2)
            q_rope(3, *qp3)
            phi_proj(3)
            if c == 0:
                nc.sync.dma_start(WoH_sb[:], WoH8)
                nc.sync.dma_start(WoL_sb[:], WoL8)

        # ================= phase B: Qg, logits, softmax, outer =================
        qg_f = small.tile([P, NHL], F32, tag="qgf")
        for h in range(NHL):
            nc.vector.tensor_reduce(qg_f[:, h:h + 1], qg_parts[:, h, :], AX, OP.add)
        qg16 = small.tile([P, NHL], F16, tag="qg16")
        nc.vector.tensor_scalar_mul(qg16[:], qg_f[:], 1.0 / S)

        psl = pr.tile([P, NST, NHL], F32, tag="r")
        for st in range(NST):
            nc.tensor.matmul(psl[:, st, :], KkT16[:, st * P:(st + 1) * P],
                             qg16[:], start=True, stop=True)
        nc.vector.tensor_copy(logits_sd[:], psl[:])

        from concourse import bass_isa

        def softmax_head(h):
            lg = logits_sd[:, :, h]                       # [128, 32] stride NHL
            pmax = small.tile([P, 1], F32, tag="pmax", name=f"pmax{h}")
            nc.vector.tensor_reduce(pmax[:], lg, AX, OP.max)
            gmax = small.tile([P, 1], F32, tag="gmax", name=f"gmax{h}")
            nc.gpsimd.partition_all_reduce(gmax[:], pmax[:], 128, bass_isa.ReduceOp.max)
            ngm = small.tile([P, 1], F32, tag="ngm", name=f"ngm{h}")
            nc.vector.tensor_scalar_mul(ngm[:], gmax[:], -1.0)
            e_sd = small.tile([P, NST], F32, tag="esd", name=f"esd{h}")
            srow = small.tile([P, 1], F32, tag="srow", name=f"srow{h}")
            nc.scalar.activation(e_sd[:], lg, ACT.Exp, bias=ngm[:], accum_out=srow[:])
            stot = small.tile([P, 1], F32, tag="stot", name=f"stot{h}")
            nc.gpsimd.partition_all_reduce(stot[:], srow[:], 128, bass_isa.ReduceOp.add)
            rcpb = small.tile([P, 1], F32, tag="rcpb", name=f"rcpb{h}")
            nc.vector.reciprocal(rcpb[:], stot[:])
            nc.vector.tensor_scalar(
                alpha_sd[:, h, :], e_sd[:], rcpb[:], float(S) / SAL,
                OP.mult, OP.mult)

        def outer_head(h):
            pso = pq.tile([P, HD], F32, tag="q", name=f"pso{h}")
            for st in range(NST):
                kka = st3.tile([P, HD], F16, tag="kka", name=f"kka{h}_{st}")
                if st % 4 == 3:
                    nc.gpsimd.tensor_scalar_mul(
                        kka[:], Kk16[:, st, :], alpha_sd[:, h, st:st + 1])
                else:
                    nc.vector.tensor_scalar_mul(
                        kka[:], Kk16[:, st, :], alpha_sd[:, h, st:st + 1])
                nc.tensor.matmul(pso[:], kka[:], v16[:, st, :],
                                 start=(st == 0), stop=(st == NST - 1))
            nc.scalar.activation(outer16[:, h, :], pso[:], ACT.Identity)

        softmax_head(0)
        softmax_head(1)
        outer_head(0)
        softmax_head(2)
        outer_head(1)
        softmax_head(3)
        outer_head(2)
        outer_head(3)

        # ================= phase C: result, ctx hi/lo, o_proj =================
        # ctx for chunk c+1 is emitted before o_proj(c): its DVE/Act chain runs
        # in the shadow of o_proj(c)'s 16 PE groups.
        dr_engine = 0

        def ctx_chunk(c):
            sl = slice(c * CS, (c + 1) * CS)
            ctxh = ctxp.tile([P, NHL, CS], F8, tag="ch", name=f"ctxh{c}")
            ctxl = ctxp.tile([P, NHL, CS], F8, tag="cl", name=f"ctxl{c}")
            for h in range(NHL):
                psr = pr.tile([P, CS], F32, tag="r", name=f"psrc{h}")
                nc.tensor.matmul(psr[:], outer16[:, h, :], QkT8[:, h, sl],
                                 start=True, stop=True)
                cx = st3.tile([P, CS], F16, tag="cx", name=f"cx{h}")
                nc.vector.tensor_mul(cx[:], phiT16[:, h, sl], psr[:])
                nc.scalar.activation(ctxh[:, h, :], cx[:], ACT.Identity)
                nc.vector.scalar_tensor_tensor(
                    ctxl[:, h, :], ctxh[:, h, :], -1.0, cx[:], OP.mult, OP.add)
            return ctxh, ctxl

        ctx_cur = ctx_chunk(0)
        for c in range(NCH):
            ctxh, ctxl = ctx_cur
            if c + 1 < NCH:
                ctx_next = ctx_chunk(c + 1)
            for st in range(4):
                stg = c * 4 + st
                ssl = slice(st * P, (st + 1) * P)
                ob = outp.tile([P, 4, CS], BF16, tag="ob")
                for nq in range(4):
                    opool = (st * 4 + nq) % 3
                    if opool == 0:
                        po = pq.tile([P, CS], F32, tag="q")
                    elif opool == 1:
                        po = pmix.tile([P, CS], F32, tag="mix")
                    else:
                        po = pphi.tile([P, CS], F32, tag="p")
                    passes = [(ctxh, WoH_sb), (ctxl, WoH_sb), (ctxh, WoL_sb)]
                    for n2 in range(2):
                        n = 0
                        for ct, wt in passes:
                            for hp in range(2):
                                nc.tensor.matmul(
                                    po[:, n2 * 256:(n2 + 1) * 256],
                                    ct[:, 2 * hp:2 * hp + 2, ssl],
                                    wt[:, hp, :, nq * CS + n2 * 256:nq * CS + (n2 + 1) * 256],
                                    start=(n == 0), stop=(n == 5), perf_mode=DR)
                                n += 1
                    if c == NCH - 1:
                        eng = dr_engine % 2      # last chunk: drain on both engines
                    else:
                        eng = 0 if dr_engine % 4 == 0 else 1
                    dr_engine += 1
                    if eng == 0:
                        nc.vector.tensor_scalar_mul(ob[:, nq, :], po[:], SCTX / SW)
                    else:
                        nc.scalar.activation(ob[:, nq, :], po[:], ACT.Identity, scale=SCTX / SW)
                    if c == NCH - 1 and st == 3:
                        # last s-tile: per-quarter DMA so the tail pipelines
                        nc.sync.dma_start(out[:, stg, nq * CS:(nq + 1) * CS], ob[:, nq, :])
                if not (c == NCH - 1 and st == 3):
                    nc.sync.dma_start(out[:, stg, :], ob[:])
            if c + 1 < NCH:
                ctx_cur = ctx_next

    nc.compile()
    return nc


def _host_prep(hidden_states, position_ids, Wq, Wk, Wv, Wo, Wphi, bphi):
    B = hidden_states.shape[0]

    def q8(a):
        return np.clip(a, -240, 240).astype(NPF8)

    def split8(a):  # fp8 hi + residual
        hi = q8(a)
        lo = q8(a - hi.astype(np.float32))
        return hi, lo

    def wlay(W, sc=True):  # [2048, M] -> [p, ko, 2, M]
        Wl = (W * SW).astype(np.float32) if sc else W
        return np.ascontiguousarray(
            Wl.reshape(KO2, 2, P, -1).transpose(2, 0, 1, 3))

    inv_freq = (1.0 / (ROPE_THETA ** (np.arange(0, HD, 2, dtype=np.float32) / HD))).astype(np.float32)
    Rm = np.zeros((P, P), dtype=np.float32)
    Rm[np.arange(64), np.arange(64) + 64] = -1.0
    Rm[np.arange(64) + 64, np.arange(64)] = 1.0
    RT_np = np.ascontiguousarray(Rm.T).astype(NPH)

    in_maps = []
    for b in range(B):
        freqs = position_ids[b].astype(np.float32)[:, None] * inv_freq[None, :]
        emb = np.concatenate([freqs, freqs], axis=1)          # [S, 128]
        cos_b = np.cos(emb) / (SX * SW)
        sin_b = np.sin(emb) / (SX * SW)
        cosqT_b = np.ascontiguousarray(cos_b.T).astype(NPH)
        sinqT_b = np.ascontiguousarray(sin_b.T).astype(NPH)
        # kcs[p, st, 0/1, d]
        kcs_b = np.ascontiguousarray(
            np.stack([cos_b.reshape(NST, P, HD), sin_b.reshape(NST, P, HD)],
                     axis=2).transpose(1, 0, 2, 3)).astype(NPH)
        xs = (hidden_states[b].T * SX).astype(np.float32)      # [HID, S]
        x8_full = q8(xs)
        dx8_full = q8(xs - x8_full.astype(np.float32))
        x8_b = np.ascontiguousarray(
            x8_full.reshape(KO2, 2, P, S).transpose(2, 0, 1, 3))
        dx8_b = np.ascontiguousarray(
            dx8_full.reshape(KO2, 2, P, S).transpose(2, 0, 1, 3))
        for g in range(4):
            sl4 = slice(g * 512, (g + 1) * 512)
            sl1 = slice(g * 128, (g + 1) * 128)
            Wq_l = q8(wlay(Wq[:, sl4]))
            Wkv_hi, Wkv_lo = split8(wlay(np.concatenate([Wk[:, sl1], Wv[:, sl1]], axis=1)))
            Wphi_hi, Wphi_lo = split8(wlay(Wphi[:, sl4]))
            # Wo [512, 2048] -> [p, hp, 2, n]
            Wo_l = (Wo[sl4, :] * SW).astype(np.float32).reshape(2, 2, P, HID).transpose(2, 0, 1, 3)
            Wo_hi, Wo_lo = split8(np.ascontiguousarray(Wo_l))
            in_maps.append({
                "x8": x8_b, "dx8": dx8_b,
                "Wq8": Wq_l, "Wkv8": Wkv_hi, "dWkv8": Wkv_lo,
                "Wphi8": Wphi_hi, "dWphi8": Wphi_lo,
                "WoH8": Wo_hi, "WoL8": Wo_lo,
                "cosqT": cosqT_b, "sinqT": sinqT_b, "kcs": kcs_b,
                "bphi_s": np.ascontiguousarray(
                    (bphi[sl4] * SAL / SCTX).astype(np.float32).reshape(NHL, P).T),
                "RT": RT_np,
            })
    return in_maps


def kernel(hidden_states, position_ids, Wq, Wk, Wv, Wo, Wphi, bphi, _trace=False):
    if "nc" not in _CACHE:
        _CACHE["nc"] = _build()
    nc = _CACHE["nc"]
    in_maps = _host_prep(np.asarray(hidden_states), np.asarray(position_ids),
                         np.asarray(Wq), np.asarray(Wk), np.asarray(Wv),
                         np.asarray(Wo), np.asarray(Wphi), np.asarray(bphi))
    res = run_bass_kernel_spmd(nc, in_maps, list(range(8)), trace=_trace)
    _CACHE["last_res"] = res
    B = hidden_states.shape[0]
    out = np.empty((B, S, HID), dtype=np.float32)
    for b in range(B):
        acc = res.results[b * 4 + 0]["out"].astype(np.float32)
        for g in range(1, 4):
            acc = acc + res.results[b * 4 + g]["out"].astype(np.float32)
        out[b] = acc.reshape(P, NST, HID).transpose(1, 0, 2).reshape(S, HID)
    return out
